# revision 20
# baseline (speedup 1.0000x reference)
"""EngramMemory kernel for 8x Trainium2 NeuronCores (Bass/Tile).

Sharding: data-parallel over the 8192-token dim (1024 tokens/core).
Per (core, slot) the bucket table is host-compacted to the <=1024 rows
actually referenced (pure layout transform; the device still performs
the full indexed gather via SWDGE dma_gather). The transposing gather
writes memory directly in [m partitions, token free] layout, which is
exactly the lhsT layout the tensor engine needs, so no on-chip
transposes are required.

Math (per token):
  y  = memory @ key_w.T            (bf16 matmul, f32 psum)
  vr = memory @ value_w.T
  gate_logit = sum(hidden*qn*kn*y) / (rms(y)*rms(hidden)*sqrt(H))
  gated = sigmoid(gate_logit) * vr/rms(vr) * vn
  out = silu(gated*conv_w[:,2] + conv_b) + gated
"""

import os
import sys

import numpy as np

for _p in ("/opt/trn_rl_repo", "/opt/pypackages"):
    if os.path.isdir(_p) and _p not in sys.path:
        sys.path.insert(0, _p)

import concourse.bass as bass
import concourse.bacc as bacc
import concourse.mybir as mybir
import concourse.tile as tile
from concourse import library_config
from concourse.bass_utils import run_bass_kernel_spmd

N, H, M = 8192, 2048, 2048
SLOTS, SLOT_DIM, BUCKETS = 8, 256, 100000
NCORES = 8
TOK = N // NCORES  # 1024 tokens per core
P = 128
NT = TOK // P  # 8 token tiles per core
MT = M // P  # 16 m-tiles (contraction)
HCH = 512  # h chunk (one psum bank)
NHC = H // HCH  # 4
CTAB_ROWS = SLOTS * TOK  # 8192 compacted rows per core
EPS = 1e-8

F32 = mybir.dt.float32
BF16 = mybir.dt.bfloat16
I16 = mybir.dt.int16
FP8 = mybir.dt.float8e4
FP8_SCALE = 64.0

_BUILT = {}


def _build_module(nt=NT, feats=("mm", "stats", "value", "gate", "hwsilu")):
    key = (nt, tuple(feats))
    if key in _BUILT:
        return _BUILT[key]
    AF = mybir.ActivationFunctionType
    OP = mybir.AluOpType

    nc = bacc.Bacc("TRN2")
    ctab = nc.dram_tensor("ctab", [CTAB_ROWS, SLOT_DIM], BF16, kind="ExternalInput")
    idx = nc.dram_tensor("idx", [P, NT, TOK // 16], I16, kind="ExternalInput")
    hid = nc.dram_tensor("hid", [TOK, H], BF16, kind="ExternalInput")
    kwT = nc.dram_tensor("kwT", [M, H], BF16, kind="ExternalInput")
    kwT8 = nc.dram_tensor("kwT8", [P, MT // 2, 2, H], FP8, kind="ExternalInput")
    vwT = nc.dram_tensor("vwT", [M, H], BF16, kind="ExternalInput")
    qnkn = nc.dram_tensor("qnkn", [1, H], BF16, kind="ExternalInput")
    vnw = nc.dram_tensor("vnw", [1, H], BF16, kind="ExternalInput")
    w2 = nc.dram_tensor("w2", [1, H], BF16, kind="ExternalInput")
    cbias = nc.dram_tensor("cbias", [1, H], BF16, kind="ExternalInput")
    out = nc.dram_tensor("out", [TOK, H], F32, kind="ExternalOutput")

    hid_r = hid.rearrange("(t p) h -> t p h", p=P)
    out_r = out.rearrange("(t p) h -> t p h", p=P)
    kwT_r = kwT.rearrange("(t p) h -> p t h", p=P)
    vwT_r = vwT.rearrange("(t p) h -> p t h", p=P)

    nc.gpsimd.load_library(library_config.attnmlp)
    with tile.TileContext(nc) as tc:
        with (
            tc.tile_pool(name="wpool", bufs=1) as wpool,
            tc.tile_pool(name="cpool", bufs=1) as cpool,
            tc.tile_pool(name="mpool", bufs=2) as mpool,
            tc.tile_pool(name="m8pool", bufs=2) as m8pool,
            tc.tile_pool(name="hpool", bufs=2) as hpool,
            tc.tile_pool(name="kpool", bufs=1) as kpool,
            tc.tile_pool(name="gpool", bufs=1) as gpool,
            tc.tile_pool(name="opool", bufs=2) as opool,
            tc.tile_pool(name="spool", bufs=2) as spool,
            tc.tile_pool(name="ypool", bufs=1, space="PSUM") as ypool,
            tc.tile_pool(name="vpool", bufs=1, space="PSUM") as vpool,
        ):
            # --- index tile first; prefetch gather for tile 0 before the
            # weight loads so PE's first matmul isn't queued behind them
            itile = cpool.tile([P, NT, TOK // 16], I16, tag="itile")
            nc.scalar.dma_start(out=itile, in_=idx[:, :, :])

            mem_tiles = {}
            NPIECE = 4
            PLEN = TOK // NPIECE

            def issue_gather(t):
                # one pool tile per gather piece so downstream matmuls depend
                # only on the piece they read (Tile tracks whole-tile writes
                # for dma_gather). Slot s lives in piece (s*P)//PLEN at offset
                # (s*P)%PLEN.
                pcs = []
                for pc in range(NPIECE):
                    mt_ = mpool.tile([P, 2, PLEN], BF16, tag=f"memT{pc}")
                    nc.gpsimd.dma_gather(
                        mt_[:],
                        ctab[:],
                        itile[:, t, pc * (PLEN // 16) : (pc + 1) * (PLEN // 16)],
                        num_idxs=PLEN,
                        num_idxs_reg=PLEN,
                        elem_size=SLOT_DIM,
                        transpose=True,
                        single_packet=False,
                    )
                    pcs.append(mt_)
                mem_tiles[t] = pcs

            def lhsT_slice(mem, pieces, s, j):
                q, off = divmod(s * P, PLEN)
                return mem[q][:, j, off : off + P]

            issue_gather(0)

            # --- resident weights, loaded per h-chunk (PE consumes per-chunk)
            fp8k = "fp8k" in feats
            if "mm" in feats and fp8k:
                kw8 = wpool.tile([P, MT // 2, 2, H], FP8, tag="kw8")
                for hc in range(NHC):
                    hs = slice(hc * HCH, (hc + 1) * HCH)
                    nc.sync.dma_start(out=kw8[:, :, :, hs], in_=kwT8[:, :, :, hs])
            elif "mm" in feats:
                kw = wpool.tile([P, MT, H], BF16, tag="kw")
                for hc in range(NHC):
                    hs = slice(hc * HCH, (hc + 1) * HCH)
                    nc.sync.dma_start(out=kw[:, :, hs], in_=kwT_r[:, :, hs])
            if "value" in feats:
                vw = wpool.tile([P, MT, H], BF16, tag="vw")
                for hc in range(NHC):
                    hs = slice(hc * HCH, (hc + 1) * HCH)
                    nc.sync.dma_start(out=vw[:, :, hs], in_=vwT_r[:, :, hs])

            # --- constants
            f_sq = "stats" in feats or "sq" in feats
            f_sh = "stats" in feats or "sh" in feats
            f_qp = "stats" in feats or "qp" in feats
            f_ttr = "stats" in feats or "ttr" in feats
            if f_qp:
                qnkn_b = cpool.tile([P, H], BF16, tag="qnkn_b")
                nc.gpsimd.dma_start(out=qnkn_b, in_=qnkn[:, :].to_broadcast([P, H]))
            if "gate" in feats:
                vn_b = cpool.tile([P, H], BF16, tag="vn_b")
                nc.gpsimd.dma_start(out=vn_b, in_=vnw[:, :].to_broadcast([P, H]))
                w2_b = cpool.tile([P, H], BF16, tag="w2_b")
                nc.gpsimd.dma_start(out=w2_b, in_=w2[:, :].to_broadcast([P, H]))
                cb_b = cpool.tile([P, H], BF16, tag="cb_b")
                nc.gpsimd.dma_start(out=cb_b, in_=cbias[:, :].to_broadcast([P, H]))
            if "stats" in feats or "gate" in feats:
                eps_t = cpool.tile([P, 1], F32, tag="eps_t")
                nc.vector.memset(eps_t, EPS)

            for t in range(nt):
                # memT[p, j, i] = ctab[lst[i], j*128+p]; i slot-major
                memT = mem_tiles.pop(t)
                mem_pieces = NPIECE
                if t + 1 < nt:
                    issue_gather(t + 1)
                if f_sh or f_qp:
                    ht = hpool.tile([P, H], BF16, tag="ht")
                    nc.scalar.dma_start(out=ht, in_=hid_r[t])

                # --- key matmul: y[n, h] += memT(s,j)[m, n].T @ kw(s,j)[m, h]
                # fp8 path: y is scaled by FP8_SCALE^2 which cancels in the
                # gate logit (t and rms_y scale identically)
                if "mm" in feats and fp8k:
                    memT8 = []
                    for pc in range(NPIECE):
                        m8_ = m8pool.tile([P, 2, PLEN], FP8, tag=f"memT8_{pc}")
                        nc.vector.tensor_scalar_mul(m8_[:], memT[pc][:], FP8_SCALE)
                        memT8.append(m8_)
                    y_ps = ypool.tile([P, H], F32, tag="y_ps")
                    for hc in range(NHC):
                        hs = slice(hc * HCH, (hc + 1) * HCH)
                        for s in range(SLOTS):
                            h, s4 = divmod(s, 4)
                            nc.tensor.matmul(
                                y_ps[:, hs],
                                lhsT=lhsT_slice(memT8, mem_pieces, s, slice(None)),  # noqa
                                rhs=kw8[:, s, :, hs],
                                start=(s == 0),
                                stop=(s == SLOTS - 1),
                                perf_mode=mybir.MatmulPerfMode.DoubleRow,
                            )
                elif "mm" in feats:
                    y_ps = ypool.tile([P, H], F32, tag="y_ps")
                    for hc in range(NHC):
                        hs = slice(hc * HCH, (hc + 1) * HCH)
                        for mt in range(MT):
                            s, j = divmod(mt, 2)
                            nc.tensor.matmul(
                                y_ps[:, hs],
                                lhsT=lhsT_slice(memT, mem_pieces, s, j),
                                rhs=kw[:, mt, hs],
                                start=(mt == 0),
                                stop=(mt == MT - 1),
                            )

                # --- stats: sy = sum(y^2), sh = sum(hid^2), tq = sum(hid*qnkn*y)
                if f_sq:
                    sy = spool.tile([P, 1], F32, tag="sy")
                    scrA = kpool.tile([P, H], BF16, tag="scrACT")
                    nc.scalar.activation(
                        out=scrA, in_=y_ps, func=AF.Square, accum_out=sy
                    )
                if f_sh:
                    sh = spool.tile([P, 1], F32, tag="sh")
                    scrA2 = kpool.tile([P, H], BF16, tag="scrACT")
                    nc.scalar.activation(
                        out=scrA2, in_=ht, func=AF.Square, accum_out=sh
                    )
                if f_qp:
                    qp = kpool.tile([P, H], BF16, tag="qp")
                    nc.vector.tensor_tensor(out=qp, in0=ht, in1=qnkn_b, op=OP.mult)
                if f_ttr:
                    tq = spool.tile([P, 1], F32, tag="tq")
                    scrD = kpool.tile([P, H], BF16, tag="scrD")
                    nc.vector.scalar_tensor_tensor(
                        out=scrD,
                        in0=y_ps,
                        scalar=1.0,
                        in1=qp,
                        op0=OP.mult,
                        op1=OP.mult,
                        accum_out=tq,
                    )

                # --- value matmul
                if "value" in feats:
                    v_ps = vpool.tile([P, H], F32, tag="v_ps")
                    for hc in range(NHC):
                        hs = slice(hc * HCH, (hc + 1) * HCH)
                        for mt in range(MT):
                            s, j = divmod(mt, 2)
                            nc.tensor.matmul(
                                v_ps[:, hs],
                                lhsT=lhsT_slice(memT, mem_pieces, s, j),
                                rhs=vw[:, mt, hs],
                                start=(mt == 0),
                                stop=(mt == MT - 1),
                            )
                if "stats" in feats and "value" in feats:
                    svp = spool.tile([P, NHC], F32, tag="svp")
                    for hc in range(NHC):
                        hs = slice(hc * HCH, (hc + 1) * HCH)
                        scrA3 = kpool.tile([P, HCH], BF16, tag="scrACT2")
                        nc.scalar.activation(
                            out=scrA3,
                            in_=v_ps[:, hs],
                            func=AF.Square,
                            accum_out=svp[:, hc : hc + 1],
                        )
                    sv = spool.tile([P, 1], F32, tag="sv")
                    nc.vector.reduce_sum(sv, svp, axis=mybir.AxisListType.X)

                if "gate" not in feats:
                    ot = opool.tile([P, H], F32, tag="ot")
                    if "value" in feats:
                        nc.scalar.activation(out=ot, in_=v_ps, func=AF.Copy)
                    elif "mm" in feats:
                        nc.scalar.activation(out=ot, in_=y_ps, func=AF.Copy)
                    else:
                        for pc in range(NPIECE):
                            nc.vector.tensor_copy(
                                out=ot[:, pc * (H // NPIECE) : (pc + 1) * (H // NPIECE)],
                                in_=memT[pc].rearrange("p j n -> p (j n)"),
                            )
                    nc.scalar.dma_start(out=out_r[t], in_=ot)
                    continue

                # --- per-token scalar lane
                rms_y = spool.tile([P, 1], F32, tag="rms_y")
                nc.scalar.activation(
                    out=rms_y, in_=sy, func=AF.Sqrt, bias=eps_t, scale=1.0 / H
                )
                rms_h = spool.tile([P, 1], F32, tag="rms_h")
                nc.scalar.activation(
                    out=rms_h, in_=sh, func=AF.Sqrt, bias=eps_t, scale=1.0 / H
                )
                rms_v = spool.tile([P, 1], F32, tag="rms_v")
                nc.scalar.activation(
                    out=rms_v, in_=sv, func=AF.Sqrt, bias=eps_t, scale=1.0 / H
                )
                den = spool.tile([P, 1], F32, tag="den")
                nc.vector.tensor_mul(den, rms_y, rms_h)
                nc.vector.tensor_scalar_mul(den, den, float(np.sqrt(H)))
                rden = spool.tile([P, 1], F32, tag="rden")
                nc.vector.reciprocal(rden, den)
                gsig = spool.tile([P, 1], F32, tag="gsig")
                nc.scalar.activation(out=gsig, in_=tq, func=AF.Sigmoid, scale=rden)
                rv = spool.tile([P, 1], F32, tag="rv")
                nc.vector.reciprocal(rv, rms_v)
                sc = spool.tile([P, 1], F32, tag="sc")
                nc.vector.tensor_mul(sc, gsig, rv)

                # --- gated = v_raw * sc * vn;  out = silu(gated*w2 + b) + gated
                gated = gpool.tile([P, H], F32, tag="gated")
                nc.vector.scalar_tensor_tensor(
                    out=gated, in0=v_ps, scalar=sc, in1=vn_b, op0=OP.mult, op1=OP.mult
                )
                ot = opool.tile([P, H], F32, tag="ot")
                nc.vector.tensor_tensor(out=ot, in0=gated, in1=w2_b, op=OP.mult)
                nc.vector.tensor_tensor(out=ot, in0=ot, in1=cb_b, op=OP.add)
                if "hwsilu" in feats:
                    silu_t = kpool.tile([P, H], F32, tag="silut")
                    nc.scalar.activation(out=silu_t, in_=ot, func=AF.Silu)
                    nc.vector.tensor_tensor(out=ot, in0=silu_t, in1=gated, op=OP.add)
                else:
                    sig_t = kpool.tile([P, H], BF16, tag="sigt")
                    nc.scalar.activation(out=sig_t, in_=ot, func=AF.Sigmoid)
                    nc.vector.tensor_tensor(out=ot, in0=ot, in1=sig_t, op=OP.mult)
                    nc.vector.tensor_tensor(out=ot, in0=ot, in1=gated, op=OP.add)
                nc.scalar.dma_start(out=out_r[t], in_=ot)

    nc.finalize()
    _BUILT[key] = nc
    return nc


def _prep_core_inputs(c, ids, tables_bf, hid_bf, kwT_bf, kwT8_i, vwT_bf, qnkn_v, vn_v, w2_v, cb_v):
    """Host-side layout prep for core c (pure data movement / index math)."""
    tok_sl = slice(c * TOK, (c + 1) * TOK)
    ids_c = ids[tok_sl]  # [TOK, SLOTS]
    ctab = np.zeros((CTAB_ROWS, SLOT_DIM), dtype=tables_bf.dtype)
    gidx = np.empty((SLOTS, TOK), dtype=np.int64)
    for s in range(SLOTS):
        u, inv = np.unique(ids_c[:, s], return_inverse=True)
        ctab[s * TOK : s * TOK + len(u)] = tables_bf[s, u]
        gidx[s] = s * TOK + inv
    # wrapped int16 idx tile: position i (= s*128 + n_local) of n-tile t holds
    # gidx[s, t*128 + n_local]; idx i lives at partition i%16, col i//16,
    # replicated into all 8 groups of 16 partitions for the 8 Q7 cores.
    lst = np.empty((NT, TOK), dtype=np.int16)
    for t in range(NT):
        for s in range(SLOTS):
            lst[t, s * P : (s + 1) * P] = gidx[s, t * P : (t + 1) * P]
    wrapped = lst.reshape(NT, TOK // 16, 16).transpose(2, 0, 1)  # [16, NT, TOK//16]
    wrapped = np.tile(wrapped, (8, 1, 1))  # [128, NT, TOK//16]
    return {
        "ctab": ctab,
        "idx": np.ascontiguousarray(wrapped),
        "hid": hid_bf[tok_sl],
        "kwT": kwT_bf,
        "kwT8": kwT8_i,
        "vwT": vwT_bf,
        "qnkn": qnkn_v,
        "vnw": vn_v,
        "w2": w2_v,
        "cbias": cb_v,
    }


def prepare_in_maps(inputs):
    import ml_dtypes

    bf16 = ml_dtypes.bfloat16
    hidden = np.asarray(inputs["hidden"], dtype=np.float32)
    ids = np.asarray(inputs["batch_ngram_bucket_ids"]).astype(np.int64)
    tables = np.asarray(inputs["tables"], dtype=np.float32)
    key_w = np.asarray(inputs["key_w"], dtype=np.float32)
    value_w = np.asarray(inputs["value_w"], dtype=np.float32)
    qn_w = np.asarray(inputs["qn_w"], dtype=np.float32)
    kn_w = np.asarray(inputs["kn_w"], dtype=np.float32)
    vn_w = np.asarray(inputs["vn_w"], dtype=np.float32)
    conv_w = np.asarray(inputs["conv_w"], dtype=np.float32)
    conv_b = np.asarray(inputs["conv_b"], dtype=np.float32)

    tables_bf = tables.astype(bf16)
    hid_bf = hidden.astype(bf16)
    kwT_bf = np.ascontiguousarray(key_w.T).astype(bf16)  # [M, H]
    fp8 = mybir.dt.np(mybir.dt.float8e4)
    # DoubleRow layout: kwT8[p, s, i, h] = key_w.T[s*256 + i*128 + p, h] * 64
    kwT8_i = np.ascontiguousarray(
        (key_w.T.reshape(MT // 2, 2, P, H).transpose(2, 0, 1, 3) * 64.0).astype(fp8)
    )
    vwT_bf = np.ascontiguousarray(value_w.T).astype(bf16)
    qnkn_v = (qn_w * kn_w).reshape(1, H).astype(bf16)
    vn_v = vn_w.reshape(1, H).astype(bf16)
    w2_v = conv_w[:, 2].reshape(1, H).astype(bf16)
    cb_v = conv_b.reshape(1, H).astype(bf16)

    return [
        _prep_core_inputs(
            c, ids, tables_bf, hid_bf, kwT_bf, kwT8_i, vwT_bf, qnkn_v, vn_v, w2_v, cb_v
        )
        for c in range(NCORES)
    ]


def kernel(**inputs) -> np.ndarray:
    nc = _build_module()
    in_maps = prepare_in_maps(inputs)
    res = run_bass_kernel_spmd(nc, in_maps, core_ids=list(range(NCORES)))
    return np.concatenate([res.results[c]["out"] for c in range(NCORES)], axis=0)
